# revision 42
# baseline (speedup 1.0000x reference)
"""Trainium2 Bass kernel for nn_Middle_Moudle_v3 (retrieval_knn).

For each episode (b, s): cosine similarity of every support spatial C-vector
against every query spatial C-vector, max over query positions.

  support_x, query_x: [8, 75, 64, 19, 19] fp32  ->  out [8, 75, 361] fp32

Sharding: data-parallel over the leading batch dim (8 episodes -> 8 cores).

Cosine similarity is scale-invariant per C-vector, so the host normalizes
each vector and int8-quantizes it with per-vector max-abs scales on BOTH
sides (scales rounded to fp16 first so codes are quantized against exactly
the scales the device applies). Support scales fold into a [N, S] fp16
matrix G applied after the max; query scales are partition-broadcast on
device and pre-multiplied into the query tiles (one 2x-rate fp16 DVE
multiply per tile). Codes are <= 127 so the fp16 GEMM accumulating in fp32
PSUM is near-exact; quantization is the only real approximation (global
rel err ~9.2e-3, elementwise max ~1.7e-2, vs the 2e-2 gate). This cuts
host->device traffic ~4x and deletes the on-device norm pipeline.

Per-core plan (75 (b,s) pairs = 38 two-pair [128, 361] tiles, partitions =
(pair, channel); tile 0 is split into per-pair [64, N] tiles DMA'd first
and cast on DVE+ACT in parallel so compute starts early; support DMAs ride
the SP queue, query DMAs the otherwise-idle Pool queue; query-scale
broadcasts are batched into 4 chunk SWDGE DMAs interleaved with the input
loop so no large enqueue blocks the early tiles):
  - ACT upcasts int8 -> fp16
  - DVE: qh = qtb * rq2 (per-tile fp16 multiply)
  - PE: per pair, 3 chunk matmuls (K=64) into one [128, 3, 512] PSUM tile
  - DVE: one batched 3-bank max-reduce per pair -> colmax [128, 3*S].
    This is the bottleneck engine: tensor_reduce has no DVE fast modes,
    so 75 pairs x 3*361 elems / 0.96 GHz = 85 us of its ~110 us busy.
  - tail: fin[mc, S] = colmax[:, m::3] * G chunk; store transposed fp16
    output [N, S] (host transposes back and upcasts)

A post-pass splits multi-wait instructions (this walrus build enforces the
one-events-slot-per-instruction ISA limit instead of splitting itself).
"""
import numpy as np

import concourse.bass as bass
import concourse.mybir as mybir
import concourse.tile as tile
from concourse.bass_utils import run_bass_kernel_spmd

F32 = mybir.dt.float32
F16 = mybir.dt.float16
BF16 = mybir.dt.bfloat16
I8 = mybir.dt.int8
B = 8          # episodes = cores
S = 75         # (b, s) pairs per core
SP = 76        # padded pairs
NT = SP // 2   # 38 two-pair tiles
C = 64         # channels
N = 361        # spatial positions (19*19)
CHUNKS = [(0, 128), (128, 128), (256, 105)]  # (offset, mc) output chunks

_ws_ctr = [0]


def _split_multi_waits(nc):
    """Move all-but-one sync wait of each instruction onto injected
    InstEventSemaphore instructions (standalone sequencer waits)."""
    for f in nc.m.functions:
        for bb in f.blocks:
            insts = list(bb.instructions)
            out = []
            changed = False
            for ins in insts:
                si = ins.sync_info
                if si is not None and len(si.on_wait) > 1:
                    waits = list(si.on_wait)
                    for w in waits[:-1]:
                        _ws_ctr[0] += 1
                        ev = mybir.InstEventSemaphore(
                            name=f"wsplit_{_ws_ctr[0]}",
                            engine=ins.engine,
                            sync_info=mybir.SyncInfo(on_wait=[w], on_update=[]),
                        )
                        out.append(ev)
                    ins.sync_info = mybir.SyncInfo(
                        on_wait=[waits[-1]], on_update=list(si.on_update)
                    )
                    changed = True
                out.append(ins)
            if changed:
                bb.instructions = out


def _build_nc():
    nc = bass.Bass(target_bir_lowering=False)
    sup_d = nc.dram_tensor("support", [S * C, N], I8, kind="ExternalInput")
    qry_d = nc.dram_tensor("query", [S * C, N], I8, kind="ExternalInput")
    qsc_d = nc.dram_tensor("qscale", [SP, N], F16, kind="ExternalInput")
    g_d = nc.dram_tensor("g", [N, S], F16, kind="ExternalInput")
    out_d = nc.dram_tensor("out", [N, S], F16, kind="ExternalOutput")

    def bcast_rows(P, np_):
        """np_ consecutive [1, N] rows of qscale, each partition-broadcast to
        C partitions (64*np_ total) in one DMA."""
        row = qsc_d[P:P + 1, :]
        return bass.AP(tensor=row.tensor, offset=row.offset,
                       ap=[[N, np_], [0, C], [1, N]])

    def bcast_chunk(j0, cnt, blk):
        """Tiles j0..j0+cnt, pair-block blk (0/1): qscale rows 2j+blk, each
        partition-broadcast to C partitions, cnt tiles per DMA (3-dim AP)."""
        row = qsc_d[2 * j0 + blk:2 * j0 + blk + 1, :]
        return bass.AP(tensor=row.tensor, offset=row.offset,
                       ap=[[0, C], [2 * N, cnt], [1, N]])

    with tile.TileContext(nc) as tc:
        with tc.tile_pool(name="inp", bufs=NT) as inp, \
             tc.tile_pool(name="work", bufs=1) as work, \
             tc.tile_pool(name="rqp", bufs=8) as rqp, \
             tc.tile_pool(name="psd", bufs=2, space="PSUM") as psd:

            colmax = work.tile([128, 3 * S], F32)  # col 3*P+m

            # tile 0 is split into per-pair [64, N] tiles whose DMAs go first
            # and whose casts run on DVE+ACT in parallel, so the first
            # matmul/reduce chain starts ~3us earlier.
            w8 = [None] * 4
            wb = [None] * 4
            for h in range(4):
                src = (sup_d, qry_d)[h % 2]
                w8[h] = work.tile([64, N], I8, tag=f"w8_{h}", name=f"w8_{h}")
                eng = nc.sync if h % 2 == 0 else nc.gpsimd
                eng.dma_start(w8[h][:], src[64 * (h // 2):64 * (h // 2) + 64, :])
            # tile-0 query-scale broadcasts go early on the sync queue
            rq0 = [None] * 2
            for e in range(2):
                rq0[e] = work.tile([64, N], F16, tag=f"rq0_{e}", name=f"rq0_{e}")
                nc.sync.dma_start(rq0[e][:], bcast_rows(e, 1))

            # input code tiles: support DMAs on the SP queue, query DMAs on
            # the (otherwise idle) Pool queue so enqueue serialization halves.
            # tile 37 holds only pair 74 (64 partitions) -- no pad pair.
            st8 = [None] * NT
            qt8 = [None] * NT
            stb = [None] * NT
            qtb = [None] * NT
            # query-scale broadcasts, batched into chunk tiles (one SWDGE DMA
            # per 64-partition block, contiguous-run descriptors)
            BCH = [(1, 4), (4, 10), (10, 19), (19, 28), (28, NT)]
            rq2c = [None] * len(BCH)

            def rq2_ap(j):
                for k, (j0, j1) in enumerate(BCH):
                    if j0 <= j < j1:
                        return rq2c[k][:, j - j0, :]

            # chunk 0 fills the idle pre-cast windows of ACT and Pool; later
            # chunks are enqueued on sync mid-loop, well before first use,
            # so no large SWDGE enqueue ever blocks the input-tile DMAs
            j0, j1 = BCH[0]
            rq2c[0] = work.tile([128, j1 - j0, N], F16, tag="rq2c0",
                                name="rq2c0")
            nc.scalar.dma_start(rq2c[0][0:C, :, :], bcast_chunk(j0, j1 - j0, 0))
            nc.gpsimd.dma_start(rq2c[0][C:128, :, :], bcast_chunk(j0, j1 - j0, 1))
            bc_after = {2: 1, 6: 2, 13: 3, 21: 4}
            for j in range(1, NT):
                pj = min(128, S * C - 128 * j)
                st8[j] = inp.tile([128, N], I8, tag="st8", name=f"st8_{j}")
                qt8[j] = inp.tile([128, N], I8, tag="qt8", name=f"qt8_{j}")
                nc.sync.dma_start(st8[j][0:pj, :], sup_d[128 * j:128 * j + pj, :])
                nc.sync.dma_start(qt8[j][0:pj, :], qry_d[128 * j:128 * j + pj, :])
                if j in bc_after:
                    k = bc_after[j]
                    j0, j1 = BCH[k]
                    rq2c[k] = work.tile([128, j1 - j0, N], F16, tag=f"rq2c{k}",
                                        name=f"rq2c{k}")
                    for blk in range(2):
                        nc.sync.dma_start(rq2c[k][C * blk:C * blk + C, :, :],
                                          bcast_chunk(j0, j1 - j0, blk))

            # folded scales are only needed by the tail -- enqueue last
            g_sb = [None] * 3
            for m, (off, mc) in enumerate(CHUNKS):
                g_sb[m] = work.tile([128, S], F16, tag=f"g{m}", name=f"g{m}")
                nc.sync.dma_start(g_sb[m][0:mc, :], g_d[off:off + mc, :])

            for h in range(4):
                wb[h] = work.tile([64, N], F16, tag=f"wb_{h}", name=f"wb_{h}")
                eng = nc.vector if h < 2 else nc.scalar
                if h < 2:
                    eng.tensor_copy(wb[h][:], w8[h][:])
                else:
                    eng.copy(wb[h][:], w8[h][:])
            qh0 = [None] * 2
            for e in range(2):
                qh0[e] = work.tile([64, N], F16, tag=f"qh0_{e}", name=f"qh0_{e}")
                nc.vector.tensor_tensor(out=qh0[e][:], in0=wb[2 * e + 1][:],
                                        in1=rq0[e][:], op=mybir.AluOpType.mult)

            def do_pair(P, lhs_t, rhs_t, e):
                dot = psd.tile([128, 3, 512], F32, tag="dot", name=f"dot{P}")
                for m, (off, mc) in enumerate(CHUNKS):
                    nc.tensor.matmul(
                        dot[0:mc, m, 0:N],
                        lhs_t[C * e:C * e + C, off:off + mc],
                        rhs_t[C * e:C * e + C, 0:N],
                        start=True, stop=True,
                    )
                nc.vector.tensor_reduce(
                    colmax[:, 3 * P:3 * P + 3], dot[:, :, 0:N],
                    axis=mybir.AxisListType.X, op=mybir.AluOpType.max,
                )

            do_pair(0, wb[0], qh0[0], 0)
            do_pair(1, wb[2], qh0[1], 0)

            for j in range(1, NT):
                pj = min(128, S * C - 128 * j)
                stb[j] = inp.tile([128, N], F16, tag="stb", name=f"stb_{j}")
                qtb[j] = inp.tile([128, N], F16, tag="qtb", name=f"qtb_{j}")
                nc.scalar.copy(stb[j][0:pj, :], st8[j][0:pj, :])
                nc.scalar.copy(qtb[j][0:pj, :], qt8[j][0:pj, :])
                # per-vector query scales: one fused fp16 multiply per tile
                # on the otherwise-idle Pool engine (~0.8us, fully parallel
                # to DVE's reduce stream, latency hidden by the qh pool)
                qh = rqp.tile([128, N], F16, tag="qh", name=f"qh_{j}")
                nc.gpsimd.tensor_tensor(out=qh[0:pj, :], in0=qtb[j][0:pj, :],
                                        in1=rq2_ap(j)[0:pj, :], op=mybir.AluOpType.mult)
                for e in range(pj // C):
                    do_pair(2 * j + e, stb[j], qh, e)

            # tail: apply folded scales, store transposed fp16 output
            # (one DMA queue per chunk so the enqueues don't serialize)
            out_q = [nc.sync, nc.gpsimd, nc.gpsimd]
            for m, (off, mc) in enumerate(CHUNKS):
                fin = work.tile([128, S], F16, tag=f"fin{m}", name=f"fin{m}")
                nc.vector.tensor_tensor(out=fin[0:mc, :], in0=colmax[0:mc, m::3],
                                        in1=g_sb[m][0:mc, :], op=mybir.AluOpType.mult)
                out_q[m].dma_start(out_d[off:off + mc, :], fin[0:mc, :])

    _split_multi_waits(nc)
    return nc


_NC_CACHE = None


def _get_nc():
    global _NC_CACHE
    if _NC_CACHE is None:
        _NC_CACHE = _build_nc()
    return _NC_CACHE


def make_in_maps(support_x, query_x):
    """Host-side fold: normalize, int8-quantize, fold all scales into G."""
    sup = np.asarray(support_x, dtype=np.float32).reshape(B, S, C, N)
    qry = np.asarray(query_x, dtype=np.float32).reshape(B, S, C, N)

    sn = np.linalg.norm(sup, axis=2, keepdims=True)   # [B,S,1,N]
    qn = np.linalg.norm(qry, axis=2, keepdims=True)
    us = sup / sn
    uq = qry / qn
    # per-vector max-abs scales, rounded to fp16 FIRST so the codes are
    # quantized against exactly the scales the device will apply
    ss = (np.abs(us).max(axis=2) / np.float32(127.0)).astype(np.float16)  # [B,S,N]
    sq = (np.abs(uq).max(axis=2) / np.float32(127.0)).astype(np.float16)  # [B,S,N]
    ss32 = ss.astype(np.float32)
    sq32 = sq.astype(np.float32)
    cs = np.rint(us / ss32[:, :, None, :]).clip(-127, 127).astype(np.int8)
    cq = np.rint(uq / sq32[:, :, None, :]).clip(-127, 127).astype(np.int8)

    # folded support scales: out[i, P] = colmax[i, P] * ss[P, i]
    g = np.ascontiguousarray(ss.transpose(0, 2, 1))   # [B, N, S] fp16

    cs = np.ascontiguousarray(cs.reshape(B, S * C, N))
    cq = np.ascontiguousarray(cq.reshape(B, S * C, N))
    sq_pad = np.zeros((B, SP, N), dtype=np.float16)   # padded so the 2-row
    sq_pad[:, :S, :] = sq                             # block DMA stays in bounds
    return [{"support": cs[b], "query": cq[b], "qscale": sq_pad[b], "g": g[b]}
            for b in range(B)]


def kernel(support_x, query_x, **_unused):
    in_maps = make_in_maps(support_x, query_x)
    nc = _get_nc()
    res = run_bass_kernel_spmd(nc, in_maps, core_ids=list(range(B)))
    out = np.stack([res.results[b]["out"].astype(np.float32).T for b in range(B)])
    return np.ascontiguousarray(out, dtype=np.float32)


# revision 43
# speedup vs baseline: 1.1864x; 1.1864x over previous
"""Trainium2 Bass kernel for nn_Middle_Moudle_v3 (retrieval_knn).

For each episode (b, s): cosine similarity of every support spatial C-vector
against every query spatial C-vector, max over query positions.

  support_x, query_x: [8, 75, 64, 19, 19] fp32  ->  out [8, 75, 361] fp32

Sharding: data-parallel over the leading batch dim (8 episodes -> 8 cores).

Cosine similarity is scale-invariant per C-vector, so the host normalizes
each vector and int8-quantizes it with per-vector max-abs scales on BOTH
sides (scales rounded to fp16 first so codes are quantized against exactly
the scales the device applies). Support scales fold into a [N, S] fp16
matrix G applied after the max; query scales are partition-broadcast on
device and pre-multiplied into the query tiles (one 2x-rate fp16 DVE
multiply per tile). Codes are <= 127 so the fp16 GEMM accumulating in fp32
PSUM is near-exact; quantization is the only real approximation (global
rel err ~9.2e-3, elementwise max ~1.7e-2, vs the 2e-2 gate). This cuts
host->device traffic ~4x and deletes the on-device norm pipeline.

Per-core plan (75 (b,s) pairs = 38 two-pair [128, 361] tiles, partitions =
(pair, channel); tile 0 is split into per-pair [64, N] tiles DMA'd first
and cast on DVE+ACT in parallel so compute starts early; support DMAs ride
the SP queue, query DMAs the otherwise-idle Pool queue; query-scale
broadcasts are batched into 4 chunk SWDGE DMAs interleaved with the input
loop so no large enqueue blocks the early tiles):
  - ACT upcasts int8 -> fp16
  - Pool: qh = qtb * rq2 (per-tile fp16 multiply, parallel to DVE)
  - PE: per pair, 3 chunk matmuls (K=64) into one [128, 3, 512] PSUM tile
  - DVE: one batched 3-bank max-reduce per pair -> colmax [128, 3*S].
    This is the bottleneck engine: tensor_reduce has no DVE fast modes,
    so 75 pairs x 3*361 elems / 0.96 GHz = 85 us of its ~100 us busy.
    (3 PSUM banks per reduce is the sweet spot: a 4-bank AP degrades to
    1.25 cyc/elem and loses outright.)
  - tail: fin[mc, S] = colmax[:, m::3] * G chunk; store transposed fp16
    output [N, S] (host transposes back and upcasts)

A post-pass splits multi-wait instructions (this walrus build enforces the
one-events-slot-per-instruction ISA limit instead of splitting itself).
"""
import numpy as np

import concourse.bass as bass
import concourse.mybir as mybir
import concourse.tile as tile
from concourse.bass_utils import run_bass_kernel_spmd

F32 = mybir.dt.float32
F16 = mybir.dt.float16
BF16 = mybir.dt.bfloat16
I8 = mybir.dt.int8
B = 8          # episodes = cores
S = 75         # (b, s) pairs per core
SP = 76        # padded pairs
NT = SP // 2   # 38 two-pair tiles
C = 64         # channels
N = 361        # spatial positions (19*19)
CHUNKS = [(0, 128), (128, 128), (256, 105)]  # (offset, mc) output chunks

_ws_ctr = [0]


def _split_multi_waits(nc):
    """Move all-but-one sync wait of each instruction onto injected
    InstEventSemaphore instructions (standalone sequencer waits)."""
    for f in nc.m.functions:
        for bb in f.blocks:
            insts = list(bb.instructions)
            out = []
            changed = False
            for ins in insts:
                si = ins.sync_info
                if si is not None and len(si.on_wait) > 1:
                    waits = list(si.on_wait)
                    for w in waits[:-1]:
                        _ws_ctr[0] += 1
                        ev = mybir.InstEventSemaphore(
                            name=f"wsplit_{_ws_ctr[0]}",
                            engine=ins.engine,
                            sync_info=mybir.SyncInfo(on_wait=[w], on_update=[]),
                        )
                        out.append(ev)
                    ins.sync_info = mybir.SyncInfo(
                        on_wait=[waits[-1]], on_update=list(si.on_update)
                    )
                    changed = True
                out.append(ins)
            if changed:
                bb.instructions = out


def _build_nc():
    nc = bass.Bass(target_bir_lowering=False)
    sup_d = nc.dram_tensor("support", [S * C, N], I8, kind="ExternalInput")
    qry_d = nc.dram_tensor("query", [S * C, N], I8, kind="ExternalInput")
    qsc_d = nc.dram_tensor("qscale", [SP, N], F16, kind="ExternalInput")
    g_d = nc.dram_tensor("g", [N, S], F16, kind="ExternalInput")
    out_d = nc.dram_tensor("out", [N, S], F16, kind="ExternalOutput")

    def bcast_rows(P, np_):
        """np_ consecutive [1, N] rows of qscale, each partition-broadcast to
        C partitions (64*np_ total) in one DMA."""
        row = qsc_d[P:P + 1, :]
        return bass.AP(tensor=row.tensor, offset=row.offset,
                       ap=[[N, np_], [0, C], [1, N]])

    def bcast_chunk(j0, cnt, blk):
        """Tiles j0..j0+cnt, pair-block blk (0/1): qscale rows 2j+blk, each
        partition-broadcast to C partitions, cnt tiles per DMA (3-dim AP)."""
        row = qsc_d[2 * j0 + blk:2 * j0 + blk + 1, :]
        return bass.AP(tensor=row.tensor, offset=row.offset,
                       ap=[[0, C], [2 * N, cnt], [1, N]])

    with tile.TileContext(nc) as tc:
        with tc.tile_pool(name="inp", bufs=NT) as inp, \
             tc.tile_pool(name="work", bufs=1) as work, \
             tc.tile_pool(name="rqp", bufs=8) as rqp, \
             tc.tile_pool(name="psd", bufs=2, space="PSUM") as psd:

            colmax = work.tile([128, 3 * S], F32)  # col 3*P+m

            # tile 0 is split into per-pair [64, N] tiles whose DMAs go first
            # and whose casts run on DVE+ACT in parallel, so the first
            # matmul/reduce chain starts ~3us earlier.
            w8 = [None] * 4
            wb = [None] * 4
            for h in range(4):
                src = (sup_d, qry_d)[h % 2]
                w8[h] = work.tile([64, N], I8, tag=f"w8_{h}", name=f"w8_{h}")
                eng = nc.sync if h % 2 == 0 else nc.gpsimd
                eng.dma_start(w8[h][:], src[64 * (h // 2):64 * (h // 2) + 64, :])
            # tile-0 query-scale broadcasts go early on the sync queue
            rq0 = [None] * 2
            for e in range(2):
                rq0[e] = work.tile([64, N], F16, tag=f"rq0_{e}", name=f"rq0_{e}")
                nc.sync.dma_start(rq0[e][:], bcast_rows(e, 1))

            # input code tiles: support DMAs on the SP queue, query DMAs on
            # the (otherwise idle) Pool queue so enqueue serialization halves.
            # tile 37 holds only pair 74 (64 partitions) -- no pad pair.
            st8 = [None] * NT
            qt8 = [None] * NT
            stb = [None] * NT
            qtb = [None] * NT
            # query-scale broadcasts, batched into chunk tiles (one SWDGE DMA
            # per 64-partition block, contiguous-run descriptors)
            BCH = [(1, 4), (4, 10), (10, 19), (19, 28), (28, NT)]
            rq2c = [None] * len(BCH)

            def rq2_ap(j):
                for k, (j0, j1) in enumerate(BCH):
                    if j0 <= j < j1:
                        return rq2c[k][:, j - j0, :]

            # chunk 0 fills the idle pre-cast windows of ACT and Pool; later
            # chunks are enqueued on sync mid-loop, well before first use,
            # so no large SWDGE enqueue ever blocks the input-tile DMAs
            j0, j1 = BCH[0]
            rq2c[0] = work.tile([128, j1 - j0, N], F16, tag="rq2c0",
                                name="rq2c0")
            nc.scalar.dma_start(rq2c[0][0:C, :, :], bcast_chunk(j0, j1 - j0, 0))
            nc.gpsimd.dma_start(rq2c[0][C:128, :, :], bcast_chunk(j0, j1 - j0, 1))
            bc_after = {2: 1, 6: 2, 13: 3, 21: 4}
            for j in range(1, NT):
                pj = min(128, S * C - 128 * j)
                st8[j] = inp.tile([128, N], I8, tag="st8", name=f"st8_{j}")
                qt8[j] = inp.tile([128, N], I8, tag="qt8", name=f"qt8_{j}")
                nc.sync.dma_start(st8[j][0:pj, :], sup_d[128 * j:128 * j + pj, :])
                nc.sync.dma_start(qt8[j][0:pj, :], qry_d[128 * j:128 * j + pj, :])
                if j in bc_after:
                    k = bc_after[j]
                    j0, j1 = BCH[k]
                    rq2c[k] = work.tile([128, j1 - j0, N], F16, tag=f"rq2c{k}",
                                        name=f"rq2c{k}")
                    for blk in range(2):
                        nc.sync.dma_start(rq2c[k][C * blk:C * blk + C, :, :],
                                          bcast_chunk(j0, j1 - j0, blk))

            # folded scales are only needed by the tail -- enqueue last
            g_sb = [None] * 3
            for m, (off, mc) in enumerate(CHUNKS):
                g_sb[m] = work.tile([128, S], F16, tag=f"g{m}", name=f"g{m}")
                nc.sync.dma_start(g_sb[m][0:mc, :], g_d[off:off + mc, :])

            for h in range(4):
                wb[h] = work.tile([64, N], F16, tag=f"wb_{h}", name=f"wb_{h}")
                eng = nc.vector if h < 2 else nc.scalar
                if h < 2:
                    eng.tensor_copy(wb[h][:], w8[h][:])
                else:
                    eng.copy(wb[h][:], w8[h][:])
            qh0 = [None] * 2
            for e in range(2):
                qh0[e] = work.tile([64, N], F16, tag=f"qh0_{e}", name=f"qh0_{e}")
                nc.vector.tensor_tensor(out=qh0[e][:], in0=wb[2 * e + 1][:],
                                        in1=rq0[e][:], op=mybir.AluOpType.mult)

            def do_pair(P, lhs_t, rhs_t, e):
                dot = psd.tile([128, 3, 512], F32, tag="dot", name=f"dot{P}")
                for m, (off, mc) in enumerate(CHUNKS):
                    nc.tensor.matmul(
                        dot[0:mc, m, 0:N],
                        lhs_t[C * e:C * e + C, off:off + mc],
                        rhs_t[C * e:C * e + C, 0:N],
                        start=True, stop=True,
                    )
                nc.vector.tensor_reduce(
                    colmax[:, 3 * P:3 * P + 3], dot[:, :, 0:N],
                    axis=mybir.AxisListType.X, op=mybir.AluOpType.max,
                )

            do_pair(0, wb[0], qh0[0], 0)
            do_pair(1, wb[2], qh0[1], 0)

            for j in range(1, NT):
                pj = min(128, S * C - 128 * j)
                stb[j] = inp.tile([128, N], F16, tag="stb", name=f"stb_{j}")
                qtb[j] = inp.tile([128, N], F16, tag="qtb", name=f"qtb_{j}")
                nc.scalar.copy(stb[j][0:pj, :], st8[j][0:pj, :])
                nc.scalar.copy(qtb[j][0:pj, :], qt8[j][0:pj, :])
                # per-vector query scales: one fused fp16 multiply per tile
                # on the otherwise-idle Pool engine (~0.8us, fully parallel
                # to DVE's reduce stream, latency hidden by the qh pool)
                qh = rqp.tile([128, N], F16, tag="qh", name=f"qh_{j}")
                nc.gpsimd.tensor_tensor(out=qh[0:pj, :], in0=qtb[j][0:pj, :],
                                        in1=rq2_ap(j)[0:pj, :], op=mybir.AluOpType.mult)
                for e in range(pj // C):
                    do_pair(2 * j + e, stb[j], qh, e)

            # tail: apply folded scales, store transposed fp16 output
            # (one DMA queue per chunk so the enqueues don't serialize)
            out_q = [nc.sync, nc.gpsimd, nc.gpsimd]
            for m, (off, mc) in enumerate(CHUNKS):
                fin = work.tile([128, S], F16, tag=f"fin{m}", name=f"fin{m}")
                nc.vector.tensor_tensor(out=fin[0:mc, :], in0=colmax[0:mc, m::3],
                                        in1=g_sb[m][0:mc, :], op=mybir.AluOpType.mult)
                out_q[m].dma_start(out_d[off:off + mc, :], fin[0:mc, :])

    _split_multi_waits(nc)
    return nc


_NC_CACHE = None


def _get_nc():
    global _NC_CACHE
    if _NC_CACHE is None:
        _NC_CACHE = _build_nc()
    return _NC_CACHE


def make_in_maps(support_x, query_x):
    """Host-side fold: normalize, int8-quantize, fold all scales into G."""
    sup = np.asarray(support_x, dtype=np.float32).reshape(B, S, C, N)
    qry = np.asarray(query_x, dtype=np.float32).reshape(B, S, C, N)

    sn = np.linalg.norm(sup, axis=2, keepdims=True)   # [B,S,1,N]
    qn = np.linalg.norm(qry, axis=2, keepdims=True)
    us = sup / sn
    uq = qry / qn
    # per-vector max-abs scales, rounded to fp16 FIRST so the codes are
    # quantized against exactly the scales the device will apply
    ss = (np.abs(us).max(axis=2) / np.float32(127.0)).astype(np.float16)  # [B,S,N]
    sq = (np.abs(uq).max(axis=2) / np.float32(127.0)).astype(np.float16)  # [B,S,N]
    ss32 = ss.astype(np.float32)
    sq32 = sq.astype(np.float32)
    cs = np.rint(us / ss32[:, :, None, :]).clip(-127, 127).astype(np.int8)
    cq = np.rint(uq / sq32[:, :, None, :]).clip(-127, 127).astype(np.int8)

    # folded support scales: out[i, P] = colmax[i, P] * ss[P, i]
    g = np.ascontiguousarray(ss.transpose(0, 2, 1))   # [B, N, S] fp16

    cs = np.ascontiguousarray(cs.reshape(B, S * C, N))
    cq = np.ascontiguousarray(cq.reshape(B, S * C, N))
    sq_pad = np.zeros((B, SP, N), dtype=np.float16)   # padded so the 2-row
    sq_pad[:, :S, :] = sq                             # block DMA stays in bounds
    return [{"support": cs[b], "query": cq[b], "qscale": sq_pad[b], "g": g[b]}
            for b in range(B)]


def kernel(support_x, query_x, **_unused):
    in_maps = make_in_maps(support_x, query_x)
    nc = _get_nc()
    res = run_bass_kernel_spmd(nc, in_maps, core_ids=list(range(B)))
    out = np.stack([res.results[b]["out"].astype(np.float32).T for b in range(B)])
    return np.ascontiguousarray(out, dtype=np.float32)
